# revision 65
# baseline (speedup 1.0000x reference)
"""Multi-head attention Trainium2 kernel, 8-core SPMD.

Sharding: 16 (batch, head) pairs over 8 cores -> each core computes 2 heads
of one batch and returns a partial [N, D] output (bf16); host sums 4
partials per batch in fp32.

Per-core dataflow (all layouts transposed, q/m on free dims so softmax'
normalization can be deferred):
  XT loaded directly: q/k/v are pre-transposed on host to [D, N], so the
  SBUF [128, DC, N] layout comes from a plain strided DMA (no xbar
  transpose).  Weights stream on the scalar queue in parallel.
  QT/KT/VT = W.T @ XT                  [2*HS, N] per head pair (scale folded
                                       into Wq on host)
  S^T[m,q] = KT_h.T @ QT_h             PSUM fp32, per m-chunk of 128
  P^T = exp(S^T)                       ACT, -> SBUF bf16 (no max subtraction:
                                       logits are O(6) by construction)
  O^T[65,q] = [V_h | 1].T @ P^T        PSUM accumulate over m; row 64 = row
                                       sums r[q] (ones-column trick)
  U = O^T -> SBUF; Un = U[0:64] / r    (recip + partition broadcast)
  out[q,:] += Un_h.T @ Wp_h            accumulated over both heads in PSUM

A run of identity transposes right after setup keeps the PE busy while the
first DMAs land so the p-state ramp (0.65 -> 1.2 -> 2.4 GHz after 3us of
continuous work) completes before the projection matmuls start.
"""

import os
import sys

import numpy as np

sys.path.insert(0, "/opt/trn_rl_repo")

import ml_dtypes
from contextlib import ExitStack

import concourse.bass as bass
import concourse.mybir as mybir
import concourse.tile as tile
from concourse import bacc
from concourse.bass_utils import run_bass_kernel_spmd
from concourse.masks import make_identity

B, N, D, H, HS = 2, 2048, 512, 8, 64
NCORES = 8
BF16 = mybir.dt.bfloat16
FP32 = mybir.dt.float32
nbf16 = ml_dtypes.bfloat16

DC = D // 128  # 4 d-chunks
MC = N // 128  # 16 m-chunks
QH = 2  # q halves
QW = N // QH  # 1024 q per chunk
# identity transposes to hold the PE p-state ramp until k0+q0 land
# (~263ns each: the ident reload serializes with the transpose)
WARMUP = 42


def build_nc(finalize=True, repeat=1):
    # 64KB/partition descriptor carveout: dynamic DMAs serialize on scratch
    # reuse with the default 16KB; a bigger ring lets transfers overlap
    nc = bacc.Bacc(dynamic_dma_scratch_size=65536)
    # host-pre-transposed activations [D, N]: strided direct2d loads (512 x
    # 2KB descriptors) run concurrently under the big carveout; xbar
    # transposes would each hog the carveout and serialize
    xq = nc.dram_tensor("xq", [D, N], BF16, kind="ExternalInput")
    xk = nc.dram_tensor("xk", [D, N], BF16, kind="ExternalInput")
    xv = nc.dram_tensor("xv", [D, N], BF16, kind="ExternalInput")
    wq = nc.dram_tensor("wq", [D, 128], BF16, kind="ExternalInput")
    wk = nc.dram_tensor("wk", [D, 128], BF16, kind="ExternalInput")
    wv = nc.dram_tensor("wv", [D, 128], BF16, kind="ExternalInput")
    wp = nc.dram_tensor("wp", [2 * HS, D], BF16, kind="ExternalInput")
    out = nc.dram_tensor("out", [N, D], BF16, kind="ExternalOutput")

    with tile.TileContext(nc) as tc, ExitStack() as ctx:
        consts = ctx.enter_context(tc.tile_pool(name="consts", bufs=1))
        xt_pool = ctx.enter_context(tc.tile_pool(name="xt", bufs=1))
        proj_pool = ctx.enter_context(tc.tile_pool(name="proj", bufs=1))
        pt_pool = ctx.enter_context(tc.tile_pool(name="pt", bufs=20))
        u_pool = ctx.enter_context(tc.tile_pool(name="u", bufs=4))
        un_pool = ctx.enter_context(tc.tile_pool(name="un", bufs=4))
        rb_pool = ctx.enter_context(tc.tile_pool(name="rb", bufs=2))
        ob_pool = ctx.enter_context(tc.tile_pool(name="ob", bufs=3))
        psA = ctx.enter_context(tc.tile_pool(name="psA", bufs=2, space="PSUM"))
        psO = ctx.enter_context(tc.tile_pool(name="psO", bufs=2, space="PSUM"))

        for _rep in range(repeat):
            ident = consts.tile([128, 128], BF16)
            make_identity(nc, ident[:])

            # PE p-state warm-up: identity transposes with no DMA deps
            for _w in range(WARMUP):
                warm = psA.tile([128, 128], BF16, tag="ps", name="warm")
                nc.tensor.transpose(warm[:], ident[:], ident[:])

            # DMA issue plan: dynamic DMAs drain round-robin between the two
            # HWDGE queues, so interleave the FIFOs to realize the serial
            # order wk,wq,wv,wp,k0,q0,v0,k1,v1,q1 (weights tiny, x halves in
            # need-order).  If the bigger carveout lets transfers overlap,
            # this order is still right.  Scalar's issues finish well
            # before the first exp needs the queue.
            wq_s = consts.tile([128, DC, 128], BF16, tag="wq_s")
            wk_s = consts.tile([128, DC, 128], BF16, tag="wk_s")
            wv_s = consts.tile([128, DC, 128], BF16, tag="wv_s")
            wp_s = consts.tile([2 * HS, D], BF16, tag="wp_s")
            nc.sync.dma_start(
                out=wk_s[:], in_=wk.rearrange("(c p) h -> p c h", p=128)
            )

            # Vn: [128, head, mc, 65]; col 64 = ones (rowsum trick)
            vn = consts.tile([128, 2, MC, HS + 1], BF16, tag="vn")
            nc.gpsimd.memset(vn[:, :, :, HS : HS + 1], 1.0)
            # lhsT/rhs must share a base partition; the rowsum row lives at
            # partition HS, so put the ones row there too
            ones_row = consts.tile([HS + 1, HS], BF16, tag="ones_row")
            nc.gpsimd.memset(ones_row[HS : HS + 1, :], 1.0)

            # X strided-loaded from the host-pre-transposed [D, N] into
            # [128, dc, N]; row d -> partition d%128, chunk d//128
            xts = {}
            for name in ("q", "k", "v"):
                xts[name] = xt_pool.tile(
                    [128, DC, N], BF16, tag=f"xt_{name}", name=f"xt_{name}"
                )

            def xdma(eng, name, half):
                dram = {"q": xq, "k": xk, "v": xv}[name]
                eng.dma_start(
                    out=xts[name][:, :, half * QW : (half + 1) * QW],
                    in_=dram[:, half * QW : (half + 1) * QW].rearrange(
                        "(c p) n -> p c n", p=128
                    ),
                )

            # k0+q0 stream alone first at full bandwidth; later halves are
            # staggered in pairs behind tiny gpsimd guard-copies (WAW into
            # the DMA dest) so they never steal bandwidth from data the PE
            # needs sooner.
            xdma(nc.sync, "k", 0)
            xdma(nc.scalar, "q", 0)
            nc.scalar.dma_start(
                out=wq_s[:], in_=wq.rearrange("(c p) h -> p c h", p=128)
            )
            nc.scalar.dma_start(
                out=wv_s[:], in_=wv.rearrange("(c p) h -> p c h", p=128)
            )
            nc.scalar.dma_start(out=wp_s[:], in_=wp[:])

            def guard(dst_name, dst_col, src_name, src_col):
                nc.gpsimd.tensor_copy(
                    xts[dst_name][:, 0, dst_col : dst_col + 1],
                    xts[src_name][:, 0, src_col : src_col + 1],
                )

            guard("v", 0, "q", 0)  # v0+k1 start once q0 has landed
            xdma(nc.sync, "v", 0)
            guard("k", QW, "q", 0)
            xdma(nc.sync, "k", 1)
            guard("v", QW, "k", QW)  # v1+q1 once k1 has landed
            xdma(nc.sync, "v", 1)
            guard("q", QW, "k", QW)
            xdma(nc.sync, "q", 1)

            # projections: [2*HS, N] = sum_dc W[dc].T @ XT[dc], emitted per
            # 512-col slice so deferred units stay under ~1us of PE time
            wmap = {"q": wq_s, "k": wk_s, "v": wv_s}
            projT = {}
            for name in ("q", "k", "v"):
                projT[name] = proj_pool.tile(
                    [128, N], BF16, tag=f"projT_{name}", name=f"projT_{name}"
                )

            def emit_proj_sl(name, half, sl):
                ps = psA.tile([128, 512], FP32, tag="ps", name="ps")
                c0 = half * QW + sl * 512
                for dc in range(DC):
                    nc.tensor.matmul(
                        ps[:],
                        wmap[name][:, dc, :],
                        xts[name][:, dc, c0 : c0 + 512],
                        start=(dc == 0),
                        stop=(dc == DC - 1),
                    )
                nc.vector.tensor_copy(projT[name][:, c0 : c0 + 512], ps[:])

            def emit_vn_block(mc0, mc1):
                # V natural: transpose VT2 per m-chunk -> [m, V_h0 | V_h1]
                for mc in range(mc0, mc1):
                    pst = psA.tile([128, 128], BF16, tag="ps", name="pst")
                    nc.tensor.transpose(
                        pst[:], projT["v"][:, mc * 128 : (mc + 1) * 128], ident[:]
                    )
                    nc.vector.tensor_copy(
                        vn[:, :, mc, 0:HS],
                        pst[:].rearrange("p (b c) -> p b c", b=2),
                    )

            # only k half-0 and q half-0 gate the first attention chunk,
            # emitted in DMA-arrival order; the rest trickles through the
            # m-loop in slots matched to each transfer's landing time (None
            # = spacer slot) so no unit head-of-line blocks the PE queue
            emit_proj_sl("k", 0, 0)
            emit_proj_sl("q", 0, 0)
            emit_proj_sl("k", 0, 1)
            emit_proj_sl("q", 0, 1)
            deferred = [
                None,
                None,
                None,
                lambda: emit_proj_sl("v", 0, 0),
                lambda: emit_proj_sl("v", 0, 1),
                lambda: emit_vn_block(0, 4),
                lambda: emit_vn_block(4, 8),
                lambda: emit_proj_sl("k", 1, 0),
                lambda: emit_proj_sl("k", 1, 1),
                lambda: emit_proj_sl("v", 1, 0),
                lambda: emit_proj_sl("v", 1, 1),
                lambda: emit_vn_block(8, 12),
                lambda: emit_vn_block(12, 16),
                lambda: emit_proj_sl("q", 1, 0),
                lambda: emit_proj_sl("q", 1, 1),
            ]

            # attention + output projection — a single flat stream over
            # (qh, mc, hh) with the PV lag carried ACROSS the qh boundary so
            # the exp pipeline never drains mid-kernel
            qt2, kt2 = projT["q"], projT["k"]

            # normalization: u copies + rowsum broadcast + reciprocal +
            # scale, all split in 512-col pieces so the final projection
            # can start before the whole row is normalized
            def emit_ucopies(o_ps_, us):
                for hh in range(2):
                    u = u_pool.tile([HS + 1, QW], BF16, tag="u", name="u")
                    nc.vector.tensor_copy(u[:], o_ps_[hh][:])
                    us[hh] = u

            def emit_norm(us, un2_):
                rbs = {}
                for hh in range(2):
                    rb_ps = psO.tile([HS, QW], FP32, tag="o", name="rb_ps")
                    for sl in range(QW // 512):
                        nc.tensor.matmul(
                            rb_ps[:, sl * 512 : (sl + 1) * 512],
                            ones_row[HS : HS + 1, :],
                            us[hh][HS : HS + 1, sl * 512 : (sl + 1) * 512],
                            start=True,
                            stop=True,
                        )
                    rb = rb_pool.tile([HS, QW], FP32, tag="rb", name="rb")
                    rbs[hh] = (rb_ps, rb)
                for piece in range(2):
                    pc = slice(piece * 512, (piece + 1) * 512)
                    for hh in range(2):
                        rb_ps, rb = rbs[hh]
                        nc.vector.reciprocal_approx_fast(rb[:, pc], rb_ps[:, pc])
                    for hh in range(2):
                        _, rb = rbs[hh]
                        nc.vector.tensor_mul(
                            un2_[HS * hh : HS * hh + HS, pc],
                            us[hh][0:HS, pc],
                            rb[:, pc],
                        )

            # output projection: both heads stacked on 128 partitions — the
            # contraction itself performs the head sum.  Chunk pairs are
            # copied into a batched bf16 tile; one DMA per 256 rows.
            def emit_final_group(qh_, un2_, g, last):
                ob = ob_pool.tile([128, 2, 512], BF16, tag="ob", name="ob")
                for j in range(2):
                    c = 2 * g + j
                    if last:
                        # tail-only: alternate psA/psO for a 4-deep f_ps
                        # rotation (both pools are free after the stream)
                        pool = psA if c % 2 else psO
                        f_ps = pool.tile(
                            [128, D],
                            FP32,
                            tag="ps" if c % 2 else "o",
                            name="f_ps",
                        )
                    else:
                        # psO slots are free once the u-copies are done;
                        # keeps psA a pure S^T/exp ping-pong mid-stream
                        f_ps = psO.tile([128, D], FP32, tag="o", name="f_ps")
                    nc.tensor.matmul(
                        f_ps[:],
                        un2_[:, c * 128 : (c + 1) * 128],
                        wp_s[:],
                        start=True,
                        stop=True,
                    )
                    if last and c % 2 == 0:
                        # ACT is idle after the last exp; share with DVE
                        nc.scalar.copy(ob[:, j, :], f_ps[:])
                    else:
                        nc.vector.tensor_copy(ob[:, j, :], f_ps[:])
                base = qh_ * QW + g * 256
                nc.sync.dma_start(
                    out=out[base : base + 256, :].rearrange(
                        "(c p) d -> p c d", p=128
                    ),
                    in_=ob[:],
                )

            o_pss = {}
            un2s = {}
            n_pv = {qh: 0 for qh in range(QH)}

            def alloc_o(qh_):
                o_pss[qh_] = {
                    hh: psO.tile([HS + 1, QW], FP32, tag="o", name=f"o_ps{hh}")
                    for hh in range(2)
                }

            def schedule_tail(qh_):
                un2 = un_pool.tile([128, QW], BF16, tag="un", name="un")
                un2s[qh_] = un2
                us = {}
                if qh_ < QH - 1:
                    # next qh's PV pops resume only once alloc_o has run, so
                    # psO slots cycle o(qh) -> rb(qh) -> f(qh) -> o(qh+1) in
                    # program order; this qh's own PVs drain meanwhile
                    deferred.append(
                        lambda o_=o_pss[qh_], us_=us: emit_ucopies(o_, us_)
                    )
                    deferred.append(lambda us_=us, u_=un2: emit_norm(us_, u_))
                    for g in range(4):
                        deferred.append(
                            lambda qh__=qh_, un2_=un2, g_=g: emit_final_group(
                                qh__, un2_, g_, False
                            )
                        )
                    deferred.append(lambda qh__=qh_: alloc_o(qh__ + 1))
                else:
                    # final tail: DVE is the critical chain, so the r-row
                    # copies ride ACT (idle after the last exp), rb comes
                    # from the now-free psA banks, and the muls read the
                    # O rows straight from PSUM (no u copies at all)
                    o_ = o_pss[qh_]
                    rbs = {}
                    for hh in range(2):
                        u = u_pool.tile([HS + 1, QW], BF16, tag="u", name="u")
                        # r-row copies run on both engines in parallel
                        if hh == 0:
                            nc.vector.tensor_copy(
                                u[HS : HS + 1, :], o_[hh][HS : HS + 1, :]
                            )
                        else:
                            nc.scalar.copy(
                                u[HS : HS + 1, :], o_[hh][HS : HS + 1, :]
                            )
                        us[hh] = u
                    for hh in range(2):
                        rb_ps = psA.tile([HS, QW], FP32, tag="ps", name="rb_ps")
                        for sl in range(QW // 512):
                            nc.tensor.matmul(
                                rb_ps[:, sl * 512 : (sl + 1) * 512],
                                ones_row[HS : HS + 1, :],
                                us[hh][HS : HS + 1, sl * 512 : (sl + 1) * 512],
                                start=True,
                                stop=True,
                            )
                        rb = rb_pool.tile([HS, QW], FP32, tag="rb", name="rb")
                        rbs[hh] = (rb_ps, rb)
                    for piece in range(2):
                        pc = slice(piece * 512, (piece + 1) * 512)
                        for hh in range(2):
                            rb_ps, rb = rbs[hh]
                            nc.vector.reciprocal_approx_fast(
                                rb[:, pc], rb_ps[:, pc]
                            )
                        for hh in range(2):
                            nc.vector.tensor_mul(
                                un2[HS * hh : HS * hh + HS, pc],
                                o_[hh][0:HS, pc],
                                rbs[hh][1][:, pc],
                            )
                    for g in range(4):
                        emit_final_group(qh_, un2, g, True)

            def pv(qh_, hh, j, p_sb):
                o_ = o_pss[qh_]
                for sl in range(QW // 512):
                    nc.tensor.matmul(
                        o_[hh][:, sl * 512 : (sl + 1) * 512],
                        vn[:, hh, j, :],
                        p_sb[:, sl * 512 : (sl + 1) * 512],
                        start=(j == 0),
                        stop=(j == MC - 1),
                    )
                n_pv[qh_] += 1
                if n_pv[qh_] == 2 * MC:
                    schedule_tail(qh_)

            alloc_o(0)
            pend = []
            slot = 0
            for qh in range(QH):
                for mc in range(MC):
                    for hh in range(2):
                        hs0 = HS * hh
                        s_ps = psA.tile([128, QW], FP32, tag="ps", name="s_ps")
                        for sl in range(QW // 512):
                            nc.tensor.matmul(
                                s_ps[:, sl * 512 : (sl + 1) * 512],
                                kt2[hs0 : hs0 + HS, mc * 128 : (mc + 1) * 128],
                                qt2[
                                    hs0 : hs0 + HS,
                                    qh * QW + sl * 512 : qh * QW + (sl + 1) * 512,
                                ],
                                start=True,
                                stop=True,
                            )
                        p_sb = pt_pool.tile([128, QW], BF16, tag="p", name="p_sb")
                        nc.scalar.activation(
                            p_sb[:], s_ps[:], mybir.ActivationFunctionType.Exp
                        )
                        # pop deferred work 2 slots late so a unit whose DMA
                        # hasn't landed can't head-of-line block the PE queue
                        if deferred and slot >= 2:
                            fn = deferred.pop(0)
                            if fn is not None:
                                fn()
                        slot += 1
                        pend.append((qh, hh, mc, p_sb))
                        # taper toward the end so the final PVs don't bunch
                        # into one PE burst; rate-limit drains to 2/slot.
                        # A PV for a qh whose o_ps isn't allocated yet (its
                        # psO slots must follow the previous tail's rb) holds
                        # until the alloc unit has run.
                        remaining = 2 * MC * QH - slot
                        lag = min(8, max(1, (remaining - 2) // 2))
                        drained = 0
                        while (
                            len(pend) > lag
                            and drained < 2
                            and pend[0][0] in o_pss
                        ):
                            pv(*pend.pop(0))
                            drained += 1
            for e in pend:
                pv(*e)
            # drain any tail work not yet popped (e.g. the last qh's units)
            while deferred:
                fn = deferred.pop(0)
                if fn is not None:
                    fn()
    if finalize:
        nc.finalize()
    return nc


_NC_CACHE = None


def _get_nc():
    global _NC_CACHE
    if _NC_CACHE is None:
        _NC_CACHE = build_nc()
    return _NC_CACHE


def make_in_maps(inputs):
    query = np.asarray(inputs["query"], np.float32)
    key = np.asarray(inputs["key"], np.float32)
    value = np.asarray(inputs["value"], np.float32)
    Wq = np.asarray(inputs["Wq"], np.float32) / np.sqrt(np.float32(HS))
    Wk = np.asarray(inputs["Wk"], np.float32)
    Wv = np.asarray(inputs["Wv"], np.float32)
    Wp = np.asarray(inputs["Wp"], np.float32)

    xt = {}
    for b in range(B):
        xt[b] = {
            "xq": np.ascontiguousarray(query[b].astype(nbf16).T),
            "xk": np.ascontiguousarray(key[b].astype(nbf16).T),
            "xv": np.ascontiguousarray(value[b].astype(nbf16).T),
        }

    in_maps = []
    for c in range(NCORES):
        b = c // 4
        h0 = 2 * (c % 4)
        m = dict(xt[b])
        m.update(
            {
                "wq": np.concatenate([Wq[h0], Wq[h0 + 1]], axis=1).astype(nbf16),
                "wk": np.concatenate([Wk[h0], Wk[h0 + 1]], axis=1).astype(nbf16),
                "wv": np.concatenate([Wv[h0], Wv[h0 + 1]], axis=1).astype(nbf16),
                "wp": np.concatenate([Wp[h0], Wp[h0 + 1]], axis=0).astype(nbf16),
            }
        )
        in_maps.append(m)
    return in_maps


def kernel(query, key, value, Wq, Wk, Wv, Wp):
    in_maps = make_in_maps(
        dict(query=query, key=key, value=value, Wq=Wq, Wk=Wk, Wv=Wv, Wp=Wp)
    )
    nc = _get_nc()
    res = run_bass_kernel_spmd(nc, in_maps, list(range(NCORES)))
    out = np.zeros((B, N, D), np.float32)
    for c in range(NCORES):
        out[c // 4] += np.asarray(res.results[c]["out"], np.float32)
    return out


if __name__ == "__main__":
    d = np.load("/root/problem/work/ref.npz")
    got = kernel(
        d["query"], d["key"], d["value"], d["Wq"], d["Wk"], d["Wv"], d["Wp"]
    )
    exp = d["expected"]
    rel = np.linalg.norm(got - exp) / np.linalg.norm(exp)
    print("Relative error:", rel)
